# revision 4
# baseline (speedup 1.0000x reference)
"""Multi-head attention (b=2, t=2048, d=1024, h=16, hd=64) on 8 trn2 NeuronCores.

Sharding: core c = 4*b + g handles batch b and head-group g (4 heads,
feature columns [g*256, (g+1)*256)). QKV weights column-sharded, Wo
row-sharded (Megatron); each core returns two partial [2048, 1024]
outputs (head-pair 0 / head-pair 1 of its group) that the host sums,
plus bo.

Datapath: fp16 operands (x, Wq/Wk/Wv, Q^T, K^T, V, probs) with fp32
PSUM accumulation; context normalize + output projection in f32r.
Softmax skips max-subtraction: scores are q.k/8 with q,k ~ N(0,1).

Schedule: the PE p-state (HAM) ramps to full clock only after ~3us of
continuous execution, so the whole kernel is laid out to keep the PE
queue busy every slot of the ACT-paced attention loop:
 - V is projected token-major directly (stationary = x^T chunk,
   moving = Wv, bias via a ones-row extra contraction step), no
   transposes at all.
 - Each pass's softmax-normalize epilogue is deferred into the next
   pass as fillers; only the ct->SBUF stage copy runs at pass end.
 - Denominator reciprocal uses reciprocal_approx_fast (~5x faster).
 - Input DMAs are chunked (512-token slices of x) and need-ordered so
   the first projections start ~5us in; the remaining pre-phase work
   (V units, K/Q chunks) fills pass (0,0) with context matmuls lagged
   6 slots behind the exp stream.
"""

import numpy as np

import concourse.bass as bass
import concourse.mybir as mybir
import concourse.tile as tile
from concourse.bass_utils import run_bass_kernel_spmd

F32 = mybir.dt.float32
F32R = mybir.dt.float32r
F16 = mybir.dt.float16
EXP = mybir.ActivationFunctionType.Exp

T = 2048          # tokens per batch
D = 1024          # model dim
HG = 4            # heads per core
HD = 64           # head dim
GF = HG * HD      # 256 features per head-group
NT = T // 128     # 16 token blocks

MAX_WAITS = 1


def _split_waits(nc):
    """walrus in this container allows only one sync-wait per instruction;
    hoist extras onto same-engine NoOps immediately before the offender."""
    for f in nc.m.functions:
        for blk in f.blocks:
            insts = list(blk.instructions)
            new, changed = [], False
            for ins in insts:
                si = ins.sync_info
                waits = list(si.on_wait) if si and si.on_wait else []
                if len(waits) > MAX_WAITS:
                    changed = True
                    extra, keep = waits[:-MAX_WAITS], waits[-MAX_WAITS:]
                    for i in range(0, len(extra), MAX_WAITS):
                        new.append(mybir.InstNoOp(
                            name=f"{ins.name}-wsplit{i}",
                            engine=ins.engine,
                            sync_info=mybir.SyncInfo(
                                on_wait=extra[i:i + MAX_WAITS], on_update=[]),
                        ))
                    ins.sync_info = mybir.SyncInfo(
                        on_wait=keep,
                        on_update=list(si.on_update) if si.on_update else [])
                new.append(ins)
            if changed:
                blk.instructions = new


def _build_program():
    nc = bass.Bass("TRN2", target_bir_lowering=False, debug=False, num_devices=8)

    xT = nc.dram_tensor("xT", [D, T], F16, kind="ExternalInput")
    Wq = nc.dram_tensor("Wq", [D, GF], F16, kind="ExternalInput")
    Wk = nc.dram_tensor("Wk", [D, GF], F16, kind="ExternalInput")
    Wv = nc.dram_tensor("Wv", [D, GF], F16, kind="ExternalInput")
    Wo = nc.dram_tensor("Wo", [GF, D], F32R, kind="ExternalInput")
    bq = nc.dram_tensor("bq", [GF, 1], F32, kind="ExternalInput")
    bk = nc.dram_tensor("bk", [GF, 1], F32, kind="ExternalInput")
    bv = nc.dram_tensor("bv", [1, GF], F16, kind="ExternalInput")
    # single output holding both head-pair partials: [pair*T + t, D]
    out = nc.dram_tensor("out", [2 * T, D], F32, kind="ExternalOutput")

    with tile.TileContext(nc) as tc:
        with (
            nc.allow_low_precision(reason="fp16/f32r rounding is intentional"),
            tc.tile_pool(name="w", bufs=1) as wp,       # persistent tiles
            tc.tile_pool(name="xt", bufs=8) as xp,      # xT tiles
            tc.tile_pool(name="pt", bufs=8) as ptp,     # probs tiles
            tc.tile_pool(name="ob", bufs=3) as obp,     # out staging
            tc.tile_pool(name="ps", bufs=2, space="PSUM") as ps,    # "sp" slots
            tc.tile_pool(name="pst", bufs=2, space="PSUM") as pst,  # S tiles
            tc.tile_pool(name="psc", bufs=1, space="PSUM") as psc,  # C accum
        ):
            # ---- persistent tiles ------------------------------------------
            # V_t[sb]: token-major [128 tokens, head, 64+1]; col 64 is ones so
            # the C psum row 64 accumulates the softmax denominator.
            V_t = [wp.tile([128, HG, HD + 1], F16, tag=f"v{tb}", name=f"v{tb}")
                   for tb in range(NT)]
            for sb in range(NT):
                nc.gpsimd.memset(V_t[sb][:, :, HD:HD + 1], 1.0)

            ones_f = wp.tile([65, 128], F32, tag="ones_f")
            nc.gpsimd.memset(ones_f[:], 1.0)
            onesr = wp.tile([65, 128], F32R, tag="onesr")
            nc.vector.tensor_copy(onesr[:], ones_f[:])
            ones16 = wp.tile([1, 128], F16, tag="ones16")
            nc.gpsimd.memset(ones16[:], 1.0)

            # ---- input DMAs, need-ordered ----------------------------------
            xT_t = [xp.tile([128, T], F16, tag="xt", name=f"xt{dc}")
                    for dc in range(8)]
            Wq_t = [wp.tile([128, GF], F16, tag=f"wq{dc}", name=f"wq{dc}")
                    for dc in range(8)]
            Wk_t = [wp.tile([128, GF], F16, tag=f"wk{dc}", name=f"wk{dc}")
                    for dc in range(8)]
            Wv_t = [wp.tile([128, GF], F16, tag=f"wv{dc}", name=f"wv{dc}")
                    for dc in range(8)]

            def dma_x(tck):
                for dc in range(8):
                    nc.sync.dma_start(
                        xT_t[dc][:, tck * 512:(tck + 1) * 512],
                        xT[dc * 128:(dc + 1) * 128, tck * 512:(tck + 1) * 512])

            for dc in range(8):
                nc.sync.dma_start(Wq_t[dc][:], Wq[dc * 128:(dc + 1) * 128, :])
            bq_t, bk_t = [], []
            for fb in range(2):
                for (lst, src, nm) in ((bq_t, bq, "bq"), (bk_t, bk, "bk")):
                    b = wp.tile([128, 1], F32, tag=f"{nm}{fb}", name=f"{nm}{fb}")
                    nc.sync.dma_start(b[:], src[fb * 128:(fb + 1) * 128, :])
                    lst.append(b)
            dma_x(0)
            for dc in range(8):
                nc.sync.dma_start(Wk_t[dc][:], Wk[dc * 128:(dc + 1) * 128, :])
            dma_x(1)
            for dc in range(8):
                nc.sync.dma_start(Wv_t[dc][:], Wv[dc * 128:(dc + 1) * 128, :])
            bv_t = wp.tile([1, GF], F16, tag="bv")
            nc.sync.dma_start(bv_t[:], bv[:, :])
            dma_x(2)
            dma_x(3)
            Wo_t = []
            for pair in range(2):
                wo = wp.tile([128, D], F32R, tag=f"wo{pair}", name=f"wo{pair}")
                nc.sync.dma_start(wo[:], Wo[pair * 128:(pair + 1) * 128, :])
                Wo_t.append(wo)

            # ---- projection helpers ----------------------------------------
            QT = [wp.tile([128, T], F16, tag=f"qt{fb}", name=f"qt{fb}")
                  for fb in range(2)]
            KT = [wp.tile([128, T], F16, tag=f"kt{fb}", name=f"kt{fb}")
                  for fb in range(2)]

            def proj_group(w_t, b_t, dst, fb, tck):
                p = ps.tile([128, 512], F32, tag="sp", name="sp")
                for dc in range(8):
                    nc.tensor.matmul(
                        p[:],
                        w_t[dc][:, fb * 128:(fb + 1) * 128],
                        xT_t[dc][:, tck * 512:(tck + 1) * 512],
                        start=(dc == 0), stop=(dc == 7))
                nc.vector.tensor_scalar_add(
                    dst[fb][:, tck * 512:(tck + 1) * 512], p[:], b_t[fb])

            # token-major V for token block sb, all 4 heads, bias folded in
            # via a ones-row extra contraction step
            def vtok(sb):
                p = ps.tile([128, HG, HD], F32, tag="sp", name="sp")
                for dc in range(8):
                    nc.tensor.matmul(
                        p[:],
                        xT_t[dc][:, sb * 128:(sb + 1) * 128],
                        Wv_t[dc][:],
                        start=(dc == 0), stop=False)
                nc.tensor.matmul(
                    p[:], ones16[0:1, :], bv_t[0:1, :],
                    start=False, stop=True)
                nc.vector.tensor_copy(V_t[sb][:, :, 0:HD], p[:])

            CTn = [wp.tile([128, T], F32R, tag=f"ctn{p}", name=f"ctn{p}")
                   for p in range(2)]
            stg = [wp.tile([65, 1024], F32R, tag=f"stg{half}",
                           name=f"stg{half}") for half in range(2)]

            # pair-`pair` partial output projection for token block tb
            def out_unit(pair, tb):
                o = obp.tile([128, D], F32, tag="o", name="o")
                for nck in range(2):
                    p = ps.tile([128, 512], F32, tag="sp", name="sp")
                    nc.tensor.matmul(
                        p[:],
                        CTn[pair][:, tb * 128:(tb + 1) * 128],
                        Wo_t[pair][:, nck * 512:(nck + 1) * 512],
                        start=True, stop=True)
                    nc.vector.tensor_copy(o[:, nck * 512:(nck + 1) * 512], p[:])
                nc.sync.dma_start(
                    out[pair * T + tb * 128:pair * T + (tb + 1) * 128, :], o[:])

            # deferred epilogue filler for pass (h, half), q-chunk q:
            # broadcast denominator row, fast-reciprocal, normalize into CTn
            def ep(h, half, q):
                fb, ro, hc = h // 2, (h % 2) * 64, half * 1024
                st = stg[half]
                rp = ps.tile([128, 512], F32, tag="sp", name="sp")
                nc.tensor.matmul(
                    rp[:], onesr[64:65, :], st[64:65, q * 512:(q + 1) * 512],
                    start=True, stop=True)
                rb = wp.tile([64, 512], F32, tag=f"rb{q}", name=f"rb{q}")
                nc.vector.reciprocal(rb[:], rp[0:64, :])
                nc.vector.tensor_mul(
                    CTn[fb][ro:ro + 64, hc + q * 512:hc + (q + 1) * 512],
                    st[0:64, q * 512:(q + 1) * 512],
                    rb[:])

            # ---- attention pass --------------------------------------------
            def attn_pass(h, half, fillers, lag=1, spill=()):
                fb, ro, hc = h // 2, (h % 2) * 64, half * 1024
                ct = psc.tile([65, 1024], F32, tag="ct", name="ct")
                pts = {}

                def c_mms(j):
                    for q in range(2):
                        nc.tensor.matmul(
                            ct[:, q * 512:(q + 1) * 512],
                            V_t[j][:, h, :],
                            pts[j][:, q * 512:(q + 1) * 512],
                            start=(j == 0), stop=(j == NT - 1))

                for sb in range(NT):
                    pt = ptp.tile([128, 1024], F16, tag="pt", name="pt")
                    pts[sb] = pt
                    st = pst.tile([128, 1024], F32, tag="st", name="st")
                    for q in range(2):
                        nc.tensor.matmul(
                            st[:, q * 512:(q + 1) * 512],
                            KT[fb][ro:ro + 64, sb * 128:(sb + 1) * 128],
                            QT[fb][ro:ro + 64,
                                   hc + q * 512:hc + (q + 1) * 512],
                            start=True, stop=True)
                    nc.scalar.activation(pt[:], st[:], EXP, scale=0.125)
                    if sb < len(fillers) and fillers[sb] is not None:
                        fillers[sb]()
                    if sb - lag >= 0:
                        c_mms(sb - lag)
                si = 0
                for j in range(NT - lag, NT):
                    if si < len(spill):
                        spill[si]()
                        si += 1
                    c_mms(j)
                while si < len(spill):
                    spill[si]()
                    si += 1
                # free ct fast: stage raw C + denominator to SBUF; the
                # normalize runs as fillers in the next pass (see ep()).
                nc.vector.tensor_copy(stg[half][:], ct[:])

            # ---- pre-phase: minimum to start pass (0,0) --------------------
            proj_group(Wq_t, bq_t, QT, 0, 0)
            proj_group(Wq_t, bq_t, QT, 0, 1)
            proj_group(Wk_t, bk_t, KT, 0, 0)

            # ---- filler schedule -------------------------------------------
            def PG(w, b, dst, fb, tck):
                return lambda: proj_group(w, b, dst, fb, tck)

            def V(sb):
                return lambda: vtok(sb)

            def OU(pair, tb):
                return lambda: out_unit(pair, tb)

            def EP(h, half, q):
                return lambda: ep(h, half, q)

            Q0 = [PG(Wq_t, bq_t, QT, 0, t) for t in range(4)]
            K0 = [PG(Wk_t, bk_t, KT, 0, t) for t in range(4)]
            Q1 = [PG(Wq_t, bq_t, QT, 1, t) for t in range(4)]
            K1 = [PG(Wk_t, bk_t, KT, 1, t) for t in range(4)]

            F = {
                (0, 0): [K0[1], V(0), V(1), V(2), V(3), V(4), K0[2], V(5),
                         V(6), V(7), K0[3], V(8), Q0[2], V(9), Q0[3], V(10)],
                (0, 1): [EP(0, 0, 0), EP(0, 0, 1), Q1[0], None, K1[0], None,
                         None, Q1[1], None, None, None, None, None, None,
                         None, None],
                (1, 0): [EP(0, 1, 0), EP(0, 1, 1), K1[1], None, Q1[2], None,
                         None, K1[2], None, None, None, None, None, None,
                         None, None],
                (1, 1): [EP(1, 0, 0), EP(1, 0, 1), Q1[3], None, K1[3], None,
                         None, None, None, None, None, None, None, None,
                         None, None],
                (2, 0): [EP(1, 1, 0), EP(1, 1, 1), OU(0, 0), None, OU(0, 1),
                         None, OU(0, 2), None, OU(0, 3), None, OU(0, 4),
                         None, OU(0, 5), None, None, None],
                (2, 1): [EP(2, 0, 0), EP(2, 0, 1), OU(0, 6), None, OU(0, 7),
                         None, OU(0, 8), None, OU(0, 9), None, OU(0, 10),
                         None, OU(0, 11), None, None, None],
                (3, 0): [EP(2, 1, 0), EP(2, 1, 1), OU(0, 12), None, None,
                         OU(0, 13), None, None, OU(0, 14), None, None,
                         OU(0, 15), None, None, None, None],
                (3, 1): [EP(3, 0, 0), EP(3, 0, 1), OU(1, 0), None, OU(1, 1),
                         OU(1, 2), None, OU(1, 3), None, OU(1, 4), None,
                         OU(1, 5), None, OU(1, 6), None, OU(1, 7)],
            }
            SPILL = {(0, 0): [V(11), V(12), V(13), V(14), V(15)]}

            for h in range(HG):
                for half in range(2):
                    attn_pass(h, half, F[(h, half)],
                              lag=6 if (h, half) == (0, 0) else 1,
                              spill=SPILL.get((h, half), ()))

            # ---- tail: last epilogue + remaining pair-1 out units ----------
            ep(3, 1, 0)
            for tb in range(8, 12):
                out_unit(1, tb)
            ep(3, 1, 1)
            for tb in range(12, 16):
                out_unit(1, tb)

    _split_waits(nc)
    return nc


_NC = None


def _get_nc():
    global _NC
    if _NC is None:
        _NC = _build_program()
    return _NC


def _shard_inputs(x, Wq, bq, Wk, bk, Wv, bv, Wo):
    xTs = [np.ascontiguousarray(x[b].T).astype(np.float16) for b in range(2)]
    in_maps = []
    for core in range(8):
        b, g = divmod(core, 4)
        lo = g * GF
        in_maps.append({
            "xT": xTs[b],
            "Wq": np.ascontiguousarray(Wq[:, lo:lo + GF]).astype(np.float16),
            "Wk": np.ascontiguousarray(Wk[:, lo:lo + GF]).astype(np.float16),
            "Wv": np.ascontiguousarray(Wv[:, lo:lo + GF]).astype(np.float16),
            "Wo": np.ascontiguousarray(Wo[lo:lo + GF, :]),
            "bq": np.ascontiguousarray(bq[lo:lo + GF].reshape(GF, 1)),
            "bk": np.ascontiguousarray(bk[lo:lo + GF].reshape(GF, 1)),
            "bv": np.ascontiguousarray(
                bv[lo:lo + GF].reshape(1, GF)).astype(np.float16),
        })
    return in_maps


def run(inputs, trace=False, trace_kwargs=None):
    """Run the kernel; returns (output [2,2048,1024] f32, BassKernelResults)."""
    inputs = {k: np.asarray(v, dtype=np.float32) for k, v in inputs.items()}
    in_maps = _shard_inputs(
        inputs["x"], inputs["Wq"], inputs["bq"], inputs["Wk"], inputs["bk"],
        inputs["Wv"], inputs["bv"], inputs["Wo"])
    nc = _get_nc()
    res = run_bass_kernel_spmd(
        nc, in_maps, list(range(8)), trace=trace, **(trace_kwargs or {}))
    bo = inputs["bo"]
    out = np.empty((2, T, D), dtype=np.float32)
    for b in range(2):
        acc = None
        for g in range(4):
            part = res.results[4 * b + g]["out"]
            for pair in range(2):
                piece = part[pair * T:(pair + 1) * T]
                acc = piece.astype(np.float32).copy() if acc is None else acc + piece
        out[b] = acc + bo[None, :]
    return out, res


def kernel(**inputs):
    out, _ = run(inputs, trace=False)
    return out


# revision 11
# speedup vs baseline: 1.0046x; 1.0046x over previous
"""Multi-head attention (b=2, t=2048, d=1024, h=16, hd=64) on 8 trn2 NeuronCores.

Sharding: core c = 4*b + g handles batch b and head-group g (4 heads,
feature columns [g*256, (g+1)*256)). QKV weights column-sharded, Wo
row-sharded (Megatron); each core returns two partial [2048, 1024]
outputs (head-pair 0 / head-pair 1 of its group) that the host sums,
plus bo.

Datapath: fp16 operands (x, Wq/Wk/Wv, Q^T, K^T, V, probs) with fp32
PSUM accumulation; context normalize + output projection in f32r.
Softmax skips max-subtraction: scores are q.k/8 with q,k ~ N(0,1).

Schedule notes (PE p-state ramps to full clock only after ~3us of
continuous execution, so every stall also halves the clock for a while):
 - Input DMAs are merged (one issue per weight matrix, one per 512-token
   x slice issued on the ACT hwdge queue) — sequencer issue time, not
   bandwidth, dominated the old startup.
 - V is projected feature-major then moved token-major via DMA-XBAR
   transposes (no PE transposes, no DVE copies).
 - Each pass's softmax-normalize epilogue is deferred into the next
   pass, split so the slow [1,512] reciprocal (DVE) never sits in front
   of a PE instruction: epA = reciprocal of the staged denominator row,
   epB (2 slots later) = PE broadcast + normalize into CTn.
 - Pass order (0,0),(1,0),(0,1),(1,1),(2,0),(3,0),(2,1),(3,1): pass 2
   reuses pass 1's projections, absorbing the pre-phase overflow.
"""

import numpy as np

import concourse.bass as bass
import concourse.mybir as mybir
import concourse.tile as tile
from concourse.bass_utils import run_bass_kernel_spmd
from concourse.masks import make_identity

F32 = mybir.dt.float32
F32R = mybir.dt.float32r
F16 = mybir.dt.float16
EXP = mybir.ActivationFunctionType.Exp

T = 2048          # tokens per batch
D = 1024          # model dim
HG = 4            # heads per core
HD = 64           # head dim
GF = HG * HD      # 256 features per head-group
NT = T // 128     # 16 token blocks

MAX_WAITS = 1
DEBUG_DUMPS = False


def _split_waits(nc):
    """walrus in this container allows only one sync-wait per instruction;
    hoist extras onto same-engine NoOps immediately before the offender."""
    for f in nc.m.functions:
        for blk in f.blocks:
            insts = list(blk.instructions)
            new, changed = [], False
            for ins in insts:
                si = ins.sync_info
                waits = list(si.on_wait) if si and si.on_wait else []
                if len(waits) > MAX_WAITS:
                    changed = True
                    extra, keep = waits[:-MAX_WAITS], waits[-MAX_WAITS:]
                    for i in range(0, len(extra), MAX_WAITS):
                        new.append(mybir.InstNoOp(
                            name=f"{ins.name}-wsplit{i}",
                            engine=ins.engine,
                            sync_info=mybir.SyncInfo(
                                on_wait=extra[i:i + MAX_WAITS], on_update=[]),
                        ))
                    ins.sync_info = mybir.SyncInfo(
                        on_wait=keep,
                        on_update=list(si.on_update) if si.on_update else [])
                new.append(ins)
            if changed:
                blk.instructions = new


def _build_program():
    nc = bass.Bass("TRN2", target_bir_lowering=False, debug=False, num_devices=8)

    xT = nc.dram_tensor("xT", [D, T], F16, kind="ExternalInput")
    Wq = nc.dram_tensor("Wq", [D, GF], F16, kind="ExternalInput")
    Wk = nc.dram_tensor("Wk", [D, GF], F16, kind="ExternalInput")
    Wv = nc.dram_tensor("Wv", [D, GF], F16, kind="ExternalInput")
    Wo = nc.dram_tensor("Wo", [GF, D], F32R, kind="ExternalInput")
    bq = nc.dram_tensor("bq", [GF, 1], F32, kind="ExternalInput")
    bk = nc.dram_tensor("bk", [GF, 1], F32, kind="ExternalInput")
    bv = nc.dram_tensor("bv", [GF, 1], F32, kind="ExternalInput")
    # single output holding both head-pair partials: [pair*T + t, D]
    out = nc.dram_tensor("out", [2 * T, D], F32, kind="ExternalOutput")

    with tile.TileContext(nc) as tc:
        with (
            nc.allow_low_precision(reason="fp16/f32r rounding is intentional"),
            tc.tile_pool(name="w", bufs=1) as wp,       # persistent tiles
            tc.tile_pool(name="pt", bufs=8) as ptp,     # probs tiles
            tc.tile_pool(name="ob", bufs=3) as obp,     # out staging
            tc.tile_pool(name="ps", bufs=2, space="PSUM") as ps,    # "sp" slots
            tc.tile_pool(name="pst", bufs=2, space="PSUM") as pst,  # S tiles
            tc.tile_pool(name="psc", bufs=1, space="PSUM") as psc,  # C accum
        ):
            # ---- persistent tiles ------------------------------------------
            # V_t[sb]: token-major [128 tokens, head, 80]; col 64 is ones
            # so the C psum row 64 accumulates the softmax denominator; cols
            # 65-79 pad the head stride to 160B (DMA-XBAR dst needs 32B align)
            V_t = [wp.tile([128, HG, 80], F16, tag=f"v{tb}", name=f"v{tb}")
                   for tb in range(NT)]
            for sb in range(NT):
                nc.gpsimd.memset(V_t[sb][:, :, HD:HD + 1], 1.0)

            ones_f = wp.tile([65, 128], F32, tag="ones_f")
            nc.gpsimd.memset(ones_f[:], 1.0)
            onesr = wp.tile([65, 128], F32R, tag="onesr")
            nc.vector.tensor_copy(onesr[:], ones_f[:])

            # ---- input DMAs: merged, need-ordered --------------------------
            # weights as single [128, 8, .] tiles (dc-major), one issue each
            xT_t = wp.tile([128, 8, T], F16, tag="xt", name="xt")
            Wq_t = wp.tile([128, 8, GF], F16, tag="wq", name="wq")
            Wk_t = wp.tile([128, 8, GF], F16, tag="wk", name="wk")
            Wv_t = wp.tile([128, 8, GF], F16, tag="wv", name="wv")
            Wo_t = wp.tile([128, 2, D], F32R, tag="wo", name="wo")

            def dma_w(dst, src, nchunk, width):
                # dst [128, nchunk, width] <- src rows (c*128 + p)
                nc.sync.dma_start(
                    dst[:],
                    src[:, :].rearrange("(c p) w -> p c w", c=nchunk, p=128))

            dma_w(Wq_t, Wq, 8, GF)
            bq_t, bk_t, bv_t = [], [], []
            for fb in range(2):
                for (lst, src, nm) in ((bq_t, bq, "bq"), (bk_t, bk, "bk"),
                                       (bv_t, bv, "bv")):
                    b = wp.tile([128, 1], F32, tag=f"{nm}{fb}", name=f"{nm}{fb}")
                    nc.sync.dma_start(b[:], src[fb * 128:(fb + 1) * 128, :])
                    lst.append(b)
            # x slices on the ACT hwdge queue (parallel to SP weight loads)
            for tck in range(4):
                nc.scalar.dma_start(
                    xT_t[:, :, tck * 512:(tck + 1) * 512],
                    xT[:, tck * 512:(tck + 1) * 512].rearrange(
                        "(c p) w -> p c w", c=8, p=128))
            dma_w(Wk_t, Wk, 8, GF)
            dma_w(Wv_t, Wv, 8, GF)
            dma_w(Wo_t, Wo, 2, D)

            # ---- projection helpers ----------------------------------------
            QT = [wp.tile([128, T], F16, tag=f"qt{fb}", name=f"qt{fb}")
                  for fb in range(2)]
            KT = [wp.tile([128, T], F16, tag=f"kt{fb}", name=f"kt{fb}")
                  for fb in range(2)]
            VT = [wp.tile([128, T], F16, tag=f"vt{fb}", name=f"vt{fb}")
                  for fb in range(2)]

            def proj_group(w_t, b_t, dst, fb, tck):
                p = ps.tile([128, 512], F32, tag="sp", name="sp")
                for dc in range(8):
                    nc.tensor.matmul(
                        p[:],
                        w_t[:, dc, fb * 128:(fb + 1) * 128],
                        xT_t[:, dc, tck * 512:(tck + 1) * 512],
                        start=(dc == 0), stop=(dc == 7))
                nc.vector.tensor_scalar_add(
                    dst[fb][:, tck * 512:(tck + 1) * 512], p[:], b_t[fb])

            ident = wp.tile([128, 128], F16, tag="ident")
            make_identity(nc, ident[:])

            # move V token-major: PE transpose of a [128,128] feature block
            # (2 heads) then per-head DVE copies into V_t[sb][:, h, 0:64].
            # (DMA-XBAR would be cheaper but corrupts reads that overlap
            # concurrent SBUF writes to the same source tile.)
            def vxbar(fb, sb):
                tp = ps.tile([128, 128], F16, tag="sp", name="sp")
                nc.tensor.transpose(
                    tp[:], VT[fb][:, sb * 128:(sb + 1) * 128], ident[:])
                for hh in range(2):
                    h = fb * 2 + hh
                    nc.vector.tensor_copy(
                        V_t[sb][:, h, 0:HD],
                        tp[:, hh * 64:(hh + 1) * 64])

            CTn = [wp.tile([128, T], F32R, tag=f"ctn{p}", name=f"ctn{p}")
                   for p in range(2)]
            stg = [wp.tile([65, 1024], F32R, tag=f"stg{half}",
                           name=f"stg{half}") for half in range(2)]

            # pair-`pair` partial output projection for token block tb
            def out_unit(pair, tb):
                o = obp.tile([128, D], F32, tag="o", name="o")
                for nck in range(2):
                    p = ps.tile([128, 512], F32, tag="sp", name="sp")
                    nc.tensor.matmul(
                        p[:],
                        CTn[pair][:, tb * 128:(tb + 1) * 128],
                        Wo_t[:, pair, nck * 512:(nck + 1) * 512],
                        start=True, stop=True)
                    nc.vector.tensor_copy(o[:, nck * 512:(nck + 1) * 512], p[:])
                nc.sync.dma_start(
                    out[pair * T + tb * 128:pair * T + (tb + 1) * 128, :], o[:])

            # deferred epilogue for pass (h, half), q-chunk q, two stages:
            # epA: reciprocal of the staged denominator row (DVE only, slow)
            # epB: PE broadcast of 1/denom + normalize into CTn (>=2 slots
            #      after epA so the PE never waits on the reciprocal)
            rd_t = [wp.tile([1, 512], F32R, tag=f"rd{q}", name=f"rd{q}")
                    for q in range(2)]

            def epA(h, half, q):
                nc.vector.reciprocal(
                    rd_t[q][:], stg[half][64:65, q * 512:(q + 1) * 512])

            def epB(h, half, q):
                fb, ro, hc = h // 2, (h % 2) * 64, half * 1024
                rp = ps.tile([64, 512], F32, tag="sp", name="sp")
                nc.tensor.matmul(
                    rp[:], onesr[0:1, 0:64], rd_t[q][0:1, :],
                    start=True, stop=True)
                nc.vector.tensor_mul(
                    CTn[fb][ro:ro + 64, hc + q * 512:hc + (q + 1) * 512],
                    stg[half][0:64, q * 512:(q + 1) * 512],
                    rp[:])

            # ---- attention pass --------------------------------------------
            def attn_pass(h, half, fillers, lag=1, spill=()):
                fb, ro, hc = h // 2, (h % 2) * 64, half * 1024
                ct = psc.tile([65, 1024], F32, tag="ct", name="ct")
                pts = {}
                done = [0]

                def c_mms(j):
                    for q in range(2):
                        nc.tensor.matmul(
                            ct[:, q * 512:(q + 1) * 512],
                            V_t[j][:, h, 0:HD + 1],
                            pts[j][:, q * 512:(q + 1) * 512],
                            start=(j == 0), stop=(j == NT - 1))
                    done[0] = j + 1

                for sb in range(NT):
                    pt = ptp.tile([128, 1024], F16, tag="pt", name="pt")
                    pts[sb] = pt
                    st = pst.tile([128, 1024], F32, tag="st", name="st")
                    for q in range(2):
                        nc.tensor.matmul(
                            st[:, q * 512:(q + 1) * 512],
                            KT[fb][ro:ro + 64, sb * 128:(sb + 1) * 128],
                            QT[fb][ro:ro + 64,
                                   hc + q * 512:hc + (q + 1) * 512],
                            start=True, stop=True)
                    nc.scalar.activation(pt[:], st[:], EXP, scale=0.125)
                    if sb < len(fillers) and fillers[sb] is not None:
                        fillers[sb]()
                    if sb - lag >= done[0]:
                        c_mms(done[0])
                si = 0
                while done[0] < NT:
                    if si < len(spill):
                        spill[si]()
                        si += 1
                    c_mms(done[0])
                while si < len(spill):
                    spill[si]()
                    si += 1
                # free ct fast: stage raw C + denominator to SBUF; the
                # normalize runs as fillers in the next pass (epA/epB).
                nc.vector.tensor_copy(stg[half][:], ct[:])

            # ---- pre-phase: minimum to start pass (0,0) --------------------
            proj_group(Wq_t, bq_t, QT, 0, 0)
            proj_group(Wq_t, bq_t, QT, 0, 1)
            proj_group(Wk_t, bk_t, KT, 0, 0)

            # ---- filler schedule -------------------------------------------
            def PG(w, b, dst, fb, tck):
                return lambda: proj_group(w, b, dst, fb, tck)

            def XB(fb, *sbs):
                return lambda: [vxbar(fb, sb) for sb in sbs]

            def OU(pair, tb):
                return lambda: out_unit(pair, tb)

            def EA(h, half, q):
                return lambda: epA(h, half, q)

            def EB(h, half, q):
                return lambda: epB(h, half, q)

            def EPS(h, half):
                return [EA(h, half, 0), EA(h, half, 1),
                        EB(h, half, 0), EB(h, half, 1)]

            Q0 = [PG(Wq_t, bq_t, QT, 0, t) for t in range(4)]
            K0 = [PG(Wk_t, bk_t, KT, 0, t) for t in range(4)]
            V0 = [PG(Wv_t, bv_t, VT, 0, t) for t in range(4)]
            Q1 = [PG(Wq_t, bq_t, QT, 1, t) for t in range(4)]
            K1 = [PG(Wk_t, bk_t, KT, 1, t) for t in range(4)]
            V1 = [PG(Wv_t, bv_t, VT, 1, t) for t in range(4)]

            F = {
                (0, 0): [K0[1], V0[0], XB(0, 0, 1), XB(0, 2, 3), V0[1],
                         XB(0, 4, 5), K0[2], XB(0, 6, 7), V0[2], XB(0, 8, 9),
                         K0[3], XB(0, 10, 11), V0[3], XB(0, 12, 13),
                         XB(0, 14, 15), None],
                (1, 0): EPS(0, 0) + [Q0[2], Q0[3], V1[0], XB(1, 0, 1),
                                     XB(1, 2, 3), V1[1], XB(1, 4, 5),
                                     XB(1, 6, 7), V1[2], XB(1, 8, 9),
                                     XB(1, 10, 11), V1[3]],
                (0, 1): EPS(1, 0) + [XB(1, 12, 13), XB(1, 14, 15), Q1[0],
                                     None, K1[0], None, Q1[1], None, K1[1],
                                     None, None, None],
                (1, 1): EPS(0, 1) + [Q1[2], None, K1[2], None, Q1[3], None,
                                     K1[3], None, None, None, None, None],
                (2, 0): EPS(1, 1) + [OU(0, 0), None, OU(0, 1), None,
                                     OU(0, 2), None, OU(0, 3), None,
                                     OU(0, 4), None, OU(0, 5), None],
                (3, 0): EPS(2, 0) + [OU(0, 6), None, OU(0, 7), None,
                                     OU(0, 8), None, OU(0, 9), None,
                                     OU(0, 10), None, OU(0, 11), None],
                (2, 1): EPS(3, 0) + [OU(0, 12), None, OU(0, 13), OU(0, 14),
                                     None, OU(0, 15), None, OU(1, 0), None,
                                     OU(1, 1), OU(1, 2), OU(1, 3)],
                (3, 1): EPS(2, 1) + [OU(1, 4), None, OU(1, 5), None,
                                     OU(1, 6), None, OU(1, 7), None, None,
                                     None, None, None],
            }
            ORDER = [(0, 0), (1, 0), (0, 1), (1, 1),
                     (2, 0), (3, 0), (2, 1), (3, 1)]

            for h, half in ORDER:
                attn_pass(h, half, F[(h, half)],
                          lag=5 if (h, half) == (0, 0) else 1)

            # ---- tail: last epilogue + remaining pair-1 out units ----------
            epA(3, 1, 0)
            epA(3, 1, 1)
            epB(3, 1, 0)
            for tb in range(8, 12):
                out_unit(1, tb)
            epB(3, 1, 1)
            for tb in range(12, 16):
                out_unit(1, tb)

            if DEBUG_DUMPS:
                dq = nc.dram_tensor("dbg_qt", [2 * 128, T], F16,
                                    kind="ExternalOutput")
                dk = nc.dram_tensor("dbg_kt", [2 * 128, T], F16,
                                    kind="ExternalOutput")
                dv = nc.dram_tensor("dbg_v", [128, NT * HG * 80], F16,
                                    kind="ExternalOutput")
                dc_ = nc.dram_tensor("dbg_ctn", [2 * 128, T], F32,
                                     kind="ExternalOutput")
                for fb in range(2):
                    nc.sync.dma_start(dq[fb * 128:(fb + 1) * 128, :], QT[fb][:])
                    nc.sync.dma_start(dk[fb * 128:(fb + 1) * 128, :], KT[fb][:])
                for sb in range(NT):
                    nc.sync.dma_start(
                        dv[:, sb * 320:(sb + 1) * 320].rearrange(
                            "p (c w) -> p c w", c=HG, w=80), V_t[sb][:])
                for pr in range(2):
                    ctmp = wp.tile([128, T], F32, tag=f"dbgc{pr}",
                                   name=f"dbgc{pr}")
                    nc.vector.tensor_copy(ctmp[:], CTn[pr][:])
                    nc.sync.dma_start(dc_[pr * 128:(pr + 1) * 128, :], ctmp[:])

    _split_waits(nc)
    return nc


_NC = None


def _get_nc():
    global _NC
    if _NC is None:
        _NC = _build_program()
    return _NC


def _shard_inputs(x, Wq, bq, Wk, bk, Wv, bv, Wo):
    xTs = [np.ascontiguousarray(x[b].T).astype(np.float16) for b in range(2)]
    in_maps = []
    for core in range(8):
        b, g = divmod(core, 4)
        lo = g * GF
        in_maps.append({
            "xT": xTs[b],
            "Wq": np.ascontiguousarray(Wq[:, lo:lo + GF]).astype(np.float16),
            "Wk": np.ascontiguousarray(Wk[:, lo:lo + GF]).astype(np.float16),
            "Wv": np.ascontiguousarray(Wv[:, lo:lo + GF]).astype(np.float16),
            "Wo": np.ascontiguousarray(Wo[lo:lo + GF, :]),
            "bq": np.ascontiguousarray(bq[lo:lo + GF].reshape(GF, 1)),
            "bk": np.ascontiguousarray(bk[lo:lo + GF].reshape(GF, 1)),
            "bv": np.ascontiguousarray(bv[lo:lo + GF].reshape(GF, 1)),
        })
    return in_maps


def run(inputs, trace=False, trace_kwargs=None):
    """Run the kernel; returns (output [2,2048,1024] f32, BassKernelResults)."""
    inputs = {k: np.asarray(v, dtype=np.float32) for k, v in inputs.items()}
    in_maps = _shard_inputs(
        inputs["x"], inputs["Wq"], inputs["bq"], inputs["Wk"], inputs["bk"],
        inputs["Wv"], inputs["bv"], inputs["Wo"])
    nc = _get_nc()
    res = run_bass_kernel_spmd(
        nc, in_maps, list(range(8)), trace=trace, **(trace_kwargs or {}))
    bo = inputs["bo"]
    out = np.empty((2, T, D), dtype=np.float32)
    for b in range(2):
        acc = None
        for g in range(4):
            part = res.results[4 * b + g]["out"]
            for pair in range(2):
                piece = part[pair * T:(pair + 1) * T]
                acc = piece.astype(np.float32).copy() if acc is None else acc + piece
        out[b] = acc + bo[None, :]
    return out, res


def kernel(**inputs):
    out, _ = run(inputs, trace=False)
    return out
